# revision 21
# baseline (speedup 1.0000x reference)
"""Windowed multi-head attention (B=128 windows, N=512, C=256, H=8) on 8 TRN2 NeuronCores.

Strategy: data-parallel over windows (16 per core). Per window:
  q/k projections run as fp8e4 DoubleRow matmuls (x and W_qk in fp8, 2x PE
  throughput; host pre-scales q rows by 16*sc*64 and k rows by 4*64 to
  center fp8 exponents, the evacuation rescales by 1/64 and adds biases,
  writing q/k to bf16 at the fp8-friendly scales).  Scores stay bf16
  (fp8 DoubleRow was measured slower there: 256-column stationary loads
  dominate the halved 256-cycle streams): S^T[j, q] in PSUM f32 at 64x
  scale.  exp(S) runs on ScalarE (scale=1/64 folded in); a
  configurable number of score tiles instead use a DVE quartic
  ((a*S+b)^2+c)^2 (rel err < 3% on |S|<1.45) to offload the scalar engine.
  AV: P stationary (bf16), [v | 1]-augmented moving operand (softmax
  denominator comes out as column 32), normalize on VectorE, PE-transpose,
  proj (bf16) -> out^T.  v/proj matmuls stay bf16 for accuracy.
The emission order is software-pipelined: window w+1's qkv work is emitted
in per-head chunks during window w's heads, and each head's post-score
work is deferred by one head so the in-order DVE queue never stalls the
PE score stream.  All layout transforms happen host-side in numpy.
"""
import os
import sys

sys.path.insert(0, "/opt/trn_rl_repo")

import numpy as np
import ml_dtypes
from contextlib import ExitStack

N_CORES = 8
B, N, C = 128, 512, 256
H, HD = 8, 32
W = B // N_CORES  # windows per core

QSC, KSC, MSC = 16.0, 4.0, 64.0  # q/k fp8 centering scales, matmul scale


def make_stages(nc, pools, consts, xt8_d, xt16_d, ot_d):
    import concourse.bass as bass
    from concourse import mybir

    F32 = mybir.dt.float32
    BF16 = mybir.dt.bfloat16
    FP8 = mybir.dt.float8e4
    Exp = mybir.ActivationFunctionType.Exp
    DR = mybir.MatmulPerfMode.DoubleRow

    (xpool, x16pool, qkpool, vpool, ppool, stpool, mmout, avpool,
     recpool, apool, atpool, finpool, upool, wpool, qpool, qstpool) = pools
    wqk_sb, wv_sb, bq_sb, bv_sb, wp_sb, bp_sb, ident = consts

    # quartic exp-substitute ((a*S+b)^2 + c)^2 * sl^2 fitted on [-1.45, 1.45]
    QA, QB, QC, QSL = 0.34415693, 0.76176680, 0.42924392, 0.99639742
    Mult = mybir.AluOpType.mult
    Add = mybir.AluOpType.add
    n_quart = int(os.environ.get("KERNEL_NQUART", "1"))
    quart_set = set()
    for i in range(n_quart):
        quart_set.add(round((i + 0.5) * 16 / n_quart - 0.5) % 16)

    def stage_qkv(iv):
        """Window iv's projections.  Returns (qdr, kdr, vaug, chunks); the
        chunk closures are interleaved with the previous window's heads."""
        xw8 = xpool.tile([128, 2, 512], FP8, tag="xw8")
        nc.sync.dma_start(out=xw8, in_=xt8_d[iv])
        xw16 = x16pool.tile([128, 2, 512], BF16, tag="xw16")
        nc.sync.dma_start(out=xw16, in_=xt16_d[iv])

        qk = [qkpool.tile([128, 512], BF16, tag=f"qk{mb}", name=f"qk{mb}")
              for mb in range(6)]
        vaug = vpool.tile([128, 4, 8, 33], BF16, tag="vaug")

        def qk_chunk(mb):
            def go():
                ps = mmout.tile([128, 512], F32, tag="mm", name="qkps")
                nc.tensor.matmul(
                    ps, wqk_sb[:, :, 128 * mb:128 * mb + 128], xw8,
                    start=True, stop=True, perf_mode=DR)
                nc.vector.tensor_scalar(
                    qk[mb][:], ps, 1.0 / MSC, bq_sb[:, mb:mb + 1],
                    Mult, Add)
            return go

        def v_chunk(tp):
            def go():
                if tp == 0:
                    nc.gpsimd.memset(vaug[:, :, :, 32:33], 1.0)
                ps = mmout.tile([128, 512], F32, tag="mm", name="vps")
                for half in range(2):
                    for cb in range(2):
                        nc.tensor.matmul(
                            ps[:, 256 * half:256 * half + 256],
                            xw16[:, cb,
                                 128 * (2 * tp + half):128 * (2 * tp + half) + 128],
                            wv_sb[:, cb, :],
                            start=(cb == 0), stop=(cb == 1))
                bvb = bass.AP(tensor=bv_sb.tensor, offset=bv_sb.offset,
                              ap=[[bv_sb.ap[0][0], 128], [0, 2],
                                  [32, 8], [1, 32]])
                nc.vector.tensor_add(
                    vaug[:, 2 * tp:2 * tp + 2, :, 0:32],
                    ps.rearrange("p (t h d) -> p t h d", t=2, h=8),
                    bvb)
            return go

        chunks = ([qk_chunk(mb) for mb in range(6)]
                  + [v_chunk(0), v_chunk(1)])
        return qk, vaug, chunks

    def new_atto():
        atto_lo = apool.tile([128, 4, 128], BF16, tag="attolo", name="attolo")
        atto_hi = apool.tile([128, 4, 128], BF16, tag="attohi", name="attohi")
        return [atto_lo, atto_hi]

    def stage_head(h, qk, vaug, atto):
        """Emit scores + st-consumers (exp / quartic i1). Returns a closure
        with the rest of the head (quartic tail, AV, recip, normalize) so the
        caller can defer its emission by one head."""
        a, g = 32 * (h % 3), h // 3

        def st_tile(s):
            if qstpool is not None and (2 * h + s) in quart_set:
                return qstpool.tile([128, 2, 512], F32, tag="qst",
                                    name=f"qst{h}_{s}")
            return stpool.tile([128, 2, 512], F32, tag="st",
                               name=f"st{h}_{s}")

        st = [st_tile(0), st_tile(1)]
        nsc = 1 if os.environ.get("KERNEL_SC_CUT", "0") == "1" else 4
        for jb in range(nsc):
            nc.tensor.matmul(
                st[jb // 2][:, jb % 2, :],
                qk[3 + g][a:a + 32, 128 * jb:128 * jb + 128],
                qk[g][a:a + 32, :],
                start=True, stop=True)
        if nsc == 1:
            st[1] = st[0]
        PHDT = FP8 if os.environ.get("KERNEL_PH_FP8", "1") == "1" else BF16
        ph = ppool.tile([128, 4, 512], PHDT, tag="ph")
        us = [None, None]
        if os.environ.get("KERNEL_NO_EXP", "0") == "1":
            nc.gpsimd.memset(ph[:, 0:2, :], 0.002)
            nc.gpsimd.memset(ph[:, 2:4, :], 0.002)
        else:
            for s in range(2):
                if (2 * h + s) in quart_set:
                    # quartic exp substitute, stage i1: u = a*(S/64) + b
                    # (frees the PSUM st tile; the rest is SBUF-only)
                    u = upool.tile([128, 2, 512], BF16, tag="u")
                    nc.vector.tensor_scalar(u, st[s], QA / MSC, QB, Mult, Add)
                    us[s] = u
                else:
                    nc.scalar.activation(out=ph[:, 2 * s:2 * s + 2, :],
                                         in_=st[s], func=Exp, scale=1.0 / MSC)

        def rest():
            for s in range(2):
                if us[s] is not None:
                    # ((a*S+b)^2 + c)^2 * sl^2
                    u = us[s]
                    wt = wpool.tile([128, 2, 512], BF16, tag="w")
                    nc.vector.tensor_mul(wt, u, u)
                    qt = qpool.tile([128, 2, 512], BF16, tag="q")
                    nc.vector.tensor_scalar(qt, wt, QC, QSL, Add, Mult)
                    nc.vector.tensor_mul(ph[:, 2 * s:2 * s + 2, :], qt, qt)
            av = avpool.tile([128, 4, 33], F32, tag="avtx")
            njb = 1 if os.environ.get("KERNEL_AV_CUT", "0") == "1" else 4
            for qb in range(4):
                for jb in range(njb):
                    nc.tensor.matmul(
                        av[:, qb, :],
                        ph[:, jb, 128 * qb:128 * qb + 128],
                        vaug[:, jb, h, :],
                        start=(jb == 0), stop=(jb == njb - 1))
            rh = recpool.tile([128, 4], F32, tag="rec")
            if os.environ.get("KERNEL_RECIP", "exact") == "approx":
                nc.vector.reciprocal_approx_fast(out=rh, in_=av[:, :, 32])
            else:
                nc.vector.reciprocal(rh, av[:, :, 32])
            rb = bass.AP(tensor=rh.tensor, offset=rh.offset,
                         ap=[[rh.ap[0][0], 128], [rh.ap[1][0], 4], [0, 32]])
            nc.vector.tensor_mul(
                atto[h // 4][:, :, 32 * (h % 4):32 * (h % 4) + 32],
                av[:, :, 0:32], rb)

        return rest

    def stage_tail(iv, atto):
        at = atpool.tile([128, 2, 512], BF16, tag="at")
        for cb in range(2):
            tx = avpool.tile([128, 4, 128], BF16, tag="avtx")
            for tb in range(4):
                nc.tensor.transpose(
                    tx[:, tb, :], atto[cb][:, tb, :], ident)
            nc.vector.tensor_copy(at[:, cb, :], tx)
        for mb in range(2):
            ps = mmout.tile([128, 512], F32, tag="mm")
            for cb in range(2):
                nc.tensor.matmul(
                    ps, wp_sb[:, cb, 128 * mb:128 * mb + 128], at[:, cb, :],
                    start=(cb == 0), stop=(cb == 1))
            fin = finpool.tile([128, 512], F32, tag="fin")
            nc.vector.tensor_scalar_add(fin, ps, bp_sb[:, mb:mb + 1])
            nc.sync.dma_start(out=ot_d[iv, :, mb, :], in_=fin)

    return stage_qkv, new_atto, stage_head, stage_tail


def build_nc(n_windows=W, repeat=None):
    import concourse.bass as bass
    import concourse.tile as tile
    from concourse import bacc, mybir
    from concourse.masks import make_identity

    if repeat is None:
        repeat = int(os.environ.get("KERNEL_REPEAT", "1"))

    F32 = mybir.dt.float32
    BF16 = mybir.dt.bfloat16
    FP8 = mybir.dt.float8e4

    nc = bacc.Bacc("TRN2", target_bir_lowering=False, debug=False,
                   num_devices=N_CORES)
    xt8_d = nc.dram_tensor("xt8", [n_windows, 128, 2, 512], FP8,
                           kind="ExternalInput").ap()
    xt16_d = nc.dram_tensor("xt16", [n_windows, 128, 2, 512], BF16,
                            kind="ExternalInput").ap()
    wqk_d = nc.dram_tensor("wqk", [128, 2, 768], FP8,
                           kind="ExternalInput").ap()
    wv_d = nc.dram_tensor("wv", [128, 2, 256], BF16,
                          kind="ExternalInput").ap()
    bq_d = nc.dram_tensor("bq", [128, 6], F32, kind="ExternalInput").ap()
    bv_d = nc.dram_tensor("bv", [128, 256], F32, kind="ExternalInput").ap()
    wp_d = nc.dram_tensor("wp", [128, 2, 256], BF16, kind="ExternalInput").ap()
    bp_d = nc.dram_tensor("bp", [128, 2], F32, kind="ExternalInput").ap()
    ot_d = nc.dram_tensor("ot", [n_windows, 128, 2, 512], F32,
                          kind="ExternalOutput").ap()

    with tile.TileContext(nc) as tc, ExitStack() as ctx:
        persist = ctx.enter_context(tc.tile_pool(name="persist", bufs=1))
        xpool = ctx.enter_context(tc.tile_pool(name="xpool", bufs=3))
        x16pool = ctx.enter_context(tc.tile_pool(name="x16pool", bufs=3))
        qkpool = ctx.enter_context(tc.tile_pool(name="qkpool", bufs=3))
        vpool = ctx.enter_context(tc.tile_pool(name="vpool", bufs=3))
        ppool = ctx.enter_context(tc.tile_pool(name="ppool", bufs=3))
        # PSUM budget is 8 banks of 2KB/partition:
        #   st 2x2 + mmout 2x1 + av/tx 2x1 = 8 (KERNEL_QST=0)
        #   st 2x2 + mmout 1 + av/tx 1 + qst 1x2 = 8 (KERNEL_QST=1)
        use_qst = (os.environ.get("KERNEL_QST", "1") == "1"
                   and int(os.environ.get("KERNEL_NQUART", "1")) > 0)
        st_bufs = int(os.environ.get("KERNEL_ST_BUFS", "2"))
        mm_bufs = int(os.environ.get("KERNEL_MM_BUFS", "1" if use_qst else "2"))
        av_bufs = int(os.environ.get("KERNEL_AV_BUFS", "1" if use_qst else "2"))
        stpool = ctx.enter_context(tc.tile_pool(
            name="stpool", bufs=st_bufs, space="PSUM"))
        mmout = ctx.enter_context(tc.tile_pool(
            name="mmout", bufs=mm_bufs, space="PSUM"))
        avpool = ctx.enter_context(tc.tile_pool(
            name="avpool", bufs=av_bufs, space="PSUM"))
        qstpool = ctx.enter_context(tc.tile_pool(
            name="qstpool", bufs=1, space="PSUM")) if use_qst else None
        recpool = ctx.enter_context(tc.tile_pool(name="recpool", bufs=8))
        apool = ctx.enter_context(tc.tile_pool(name="apool", bufs=3))
        atpool = ctx.enter_context(tc.tile_pool(name="atpool", bufs=3))
        finpool = ctx.enter_context(tc.tile_pool(name="finpool", bufs=4))
        upool = ctx.enter_context(tc.tile_pool(name="upool", bufs=2))
        wpool = ctx.enter_context(tc.tile_pool(name="wpool", bufs=2))
        qpool = ctx.enter_context(tc.tile_pool(name="qpool", bufs=2))

        wqk_sb = persist.tile([128, 2, 768], FP8, tag="wqk")
        nc.sync.dma_start(out=wqk_sb, in_=wqk_d)
        wv_sb = persist.tile([128, 2, 256], BF16, tag="wv")
        nc.sync.dma_start(out=wv_sb, in_=wv_d)
        bq_sb = persist.tile([128, 6], F32, tag="bq")
        nc.sync.dma_start(out=bq_sb, in_=bq_d)
        bv_sb = persist.tile([128, 256], F32, tag="bv")
        nc.sync.dma_start(out=bv_sb, in_=bv_d)
        wp_sb = persist.tile([128, 2, 256], BF16, tag="wp")
        nc.sync.dma_start(out=wp_sb, in_=wp_d)
        bp_sb = persist.tile([128, 2], F32, tag="bp")
        nc.sync.dma_start(out=bp_sb, in_=bp_d)
        ident = persist.tile([128, 128], BF16, tag="id")
        make_identity(nc, ident)

        pools = (xpool, x16pool, qkpool, vpool, ppool, stpool, mmout,
                 avpool, recpool, apool, atpool, finpool, upool, wpool, qpool,
                 qstpool)
        consts = (wqk_sb, wv_sb, bq_sb, bv_sb, wp_sb, bp_sb, ident)
        stage_qkv, new_atto, stage_head, stage_tail = make_stages(
            nc, pools, consts, xt8_d, xt16_d, ot_d)

        def full_pass():
            defer = os.environ.get("KERNEL_DEFER", "1") == "1"
            interleave = os.environ.get("KERNEL_QKV_INTERLEAVE", "1") == "1"
            rest_q = []

            def push(r):
                if not defer:
                    r()
                    return
                rest_q.append(r)
                if len(rest_q) > 1:
                    rest_q.pop(0)()

            qk, vaug, chunks = stage_qkv(0)
            for c in chunks:
                c()
            for w in range(n_windows):
                atto = new_atto()
                nxt = stage_qkv(w + 1) if w + 1 < n_windows else None
                nchunks = nxt[2] if nxt else []
                for h in range(H):
                    push(stage_head(h, qk, vaug, atto))
                    if interleave and h < len(nchunks):
                        nchunks[h]()
                for c in (nchunks[H:] if interleave else nchunks):
                    c()
                while rest_q:
                    rest_q.pop(0)()
                stage_tail(w, atto)
                if nxt:
                    qk, vaug = nxt[0], nxt[1]

        body_passes = int(os.environ.get("KERNEL_BODY_PASSES", "1"))
        if repeat > 1:
            def rep_body(r):
                for _ in range(body_passes):
                    full_pass()
            tc.For_i_unrolled(0, repeat, 1, rep_body, max_unroll=1)
        else:
            full_pass()

    nc.compile()
    return nc


def prep_inputs(x, qkv_w, qkv_b, proj_w, proj_b, n_windows_per_core=W,
                n_cores=N_CORES):
    """Shard + lay out inputs for the per-core DRAM parameters."""
    from concourse import mybir

    FP8NP = mybir.dt.np(mybir.dt.float8e4)
    x = np.asarray(x, dtype=np.float32)
    qkv_w = np.asarray(qkv_w, dtype=np.float32)
    qkv_b = np.asarray(qkv_b, dtype=np.float32)
    proj_w = np.asarray(proj_w, dtype=np.float32)
    proj_b = np.asarray(proj_b, dtype=np.float32)

    sc = HD ** -0.5
    # q rows scaled by sc*QSC, k rows by KSC (fp8 exponent centering); the
    # fp8 matmuls run a further MSC hotter (folded into the weights), the
    # evacuation rescales by 1/MSC and adds the (sc*QSC / KSC)-scaled biases.
    qkv_w_s = qkv_w.copy()
    qkv_w_s[:C] *= sc * QSC * MSC
    qkv_w_s[C:2 * C] *= KSC * MSC
    qkv_b_s = qkv_b.copy()
    qkv_b_s[:C] *= sc * QSC
    qkv_b_s[C:2 * C] *= KSC

    # q/k feature blocks: 3 heads (96 feats) per 128-col block, zero padded,
    # so every head starts at partition offset 0/32/64.
    qpad = np.zeros((384, C), np.float32)
    kpad = np.zeros((384, C), np.float32)
    bqpad = np.zeros(768, np.float32)
    for b in range(3):
        lo, hi = 96 * b, min(96 * b + 96, C)
        qpad[128 * b:128 * b + hi - lo] = qkv_w_s[lo:hi]
        kpad[128 * b:128 * b + hi - lo] = qkv_w_s[C + lo:C + hi]
        bqpad[128 * b:128 * b + hi - lo] = qkv_b_s[lo:hi]
        bqpad[384 + 128 * b:384 + 128 * b + hi - lo] = qkv_b_s[C + lo:C + hi]
    wqk_all = np.concatenate([qpad, kpad], axis=0)  # [768, C]
    wqk = np.ascontiguousarray(
        wqk_all.reshape(768, 2, 128).transpose(2, 1, 0)).astype(FP8NP)
    wv = np.ascontiguousarray(
        qkv_w_s[2 * C:].reshape(C, 2, 128).transpose(2, 1, 0)
    ).astype(ml_dtypes.bfloat16)
    bq = np.ascontiguousarray(bqpad.reshape(6, 128).T)
    bv = np.ascontiguousarray(np.broadcast_to(qkv_b[2 * C:], (128, C)))
    wp = np.ascontiguousarray(
        proj_w.reshape(C, 2, 128).transpose(2, 1, 0)).astype(ml_dtypes.bfloat16)
    bp = np.ascontiguousarray(proj_b.reshape(2, 128).T)

    in_maps = []
    for c in range(n_cores):
        xs = x[c * n_windows_per_core:(c + 1) * n_windows_per_core]
        xt = np.ascontiguousarray(
            xs.reshape(n_windows_per_core, N, 2, 128).transpose(0, 3, 2, 1))
        in_maps.append(
            {"xt8": xt.astype(FP8NP), "xt16": xt.astype(ml_dtypes.bfloat16),
             "wqk": wqk, "wv": wv, "bq": bq, "bv": bv, "wp": wp, "bp": bp})
    return in_maps


def assemble_output(results, n_windows_per_core=W, n_cores=N_CORES):
    outs = []
    for c in range(n_cores):
        ot = results[c]["ot"]  # [W, 128, 2, 512]
        y = ot.transpose(0, 3, 2, 1).reshape(n_windows_per_core, N, C)
        outs.append(y)
    return np.ascontiguousarray(np.concatenate(outs, axis=0), dtype=np.float32)


_NC_CACHE = {}
LAST_EXEC_TIME_NS = None


def kernel(x, qkv_w, qkv_b, proj_w, proj_b):
    global LAST_EXEC_TIME_NS
    from concourse.bass_utils import run_bass_kernel_spmd

    if "nc" not in _NC_CACHE:
        _NC_CACHE["nc"] = build_nc(W, repeat=1)
    nc = _NC_CACHE["nc"]

    in_maps = prep_inputs(x, qkv_w, qkv_b, proj_w, proj_b)
    res = run_bass_kernel_spmd(nc, in_maps, core_ids=list(range(N_CORES)))
    LAST_EXEC_TIME_NS = res.exec_time_ns
    return assemble_output(res.results)


# revision 22
# speedup vs baseline: 1.1971x; 1.1971x over previous
"""Windowed multi-head attention (B=128 windows, N=512, C=256, H=8) on 8 TRN2 NeuronCores.

Strategy: data-parallel over windows (16 per core). Per window:
  q/k projections run as fp8e4 DoubleRow matmuls (x and W_qk in fp8, 2x PE
  throughput; host pre-scales q rows by 16*sc*64 and k rows by 4*64 to
  center fp8 exponents, the evacuation rescales by 1/64 and adds biases,
  writing q/k to bf16 at the fp8-friendly scales).  Scores stay bf16
  (fp8 DoubleRow was measured slower there: 256-column stationary loads
  dominate the halved 256-cycle streams): S^T[j, q] in PSUM f32 at 64x
  scale.  exp(S) runs on ScalarE (scale=1/64 folded in); a
  configurable number of score tiles instead use a DVE quartic
  ((a*S+b)^2+c)^2 (rel err < 3% on |S|<1.45) to offload the scalar engine.
  AV: P stationary (bf16), [v | 1]-augmented moving operand (softmax
  denominator comes out as column 32), normalize on VectorE, PE-transpose,
  proj (bf16) -> out^T.  v/proj matmuls stay bf16 for accuracy.
The emission order is software-pipelined: window w+1's qkv work is emitted
in per-head chunks during window w's heads, and each head's post-score
work is deferred by one head so the in-order DVE queue never stalls the
PE score stream.  All layout transforms happen host-side in numpy.
"""
import os
import sys

sys.path.insert(0, "/opt/trn_rl_repo")

import numpy as np
import ml_dtypes
from contextlib import ExitStack

N_CORES = 8
B, N, C = 128, 512, 256
H, HD = 8, 32
W = B // N_CORES  # windows per core

QSC, KSC, MSC = 16.0, 4.0, 64.0  # q/k fp8 centering scales, matmul scale


def make_stages(nc, pools, consts, xt8_d, xt16_d, ot_d):
    import concourse.bass as bass
    from concourse import mybir

    F32 = mybir.dt.float32
    BF16 = mybir.dt.bfloat16
    FP8 = mybir.dt.float8e4
    Exp = mybir.ActivationFunctionType.Exp
    DR = mybir.MatmulPerfMode.DoubleRow

    (xpool, x16pool, qkpool, vpool, ppool, stpool, mmout, avpool,
     recpool, apool, atpool, finpool, upool, wpool, qpool, qstpool) = pools
    wqk_sb, wv_sb, bq_sb, bv_sb, wp_sb, bp_sb, ident = consts

    # quartic exp-substitute ((a*S+b)^2 + c)^2 * sl^2 fitted on [-1.45, 1.45]
    QA, QB, QC, QSL = 0.34415693, 0.76176680, 0.42924392, 0.99639742
    Mult = mybir.AluOpType.mult
    Add = mybir.AluOpType.add
    n_quart = int(os.environ.get("KERNEL_NQUART", "1"))
    quart_set = set()
    for i in range(n_quart):
        quart_set.add(round((i + 0.5) * 16 / n_quart - 0.5) % 16)

    def stage_qkv(iv):
        """Window iv's projections.  Returns (qdr, kdr, vaug, chunks); the
        chunk closures are interleaved with the previous window's heads."""
        xw8 = xpool.tile([128, 2, 512], FP8, tag="xw8")
        nc.sync.dma_start(out=xw8, in_=xt8_d[iv])
        xw16 = x16pool.tile([128, 2, 512], BF16, tag="xw16")
        nc.sync.dma_start(out=xw16, in_=xt16_d[iv])

        qk = [qkpool.tile([128, 512], BF16, tag=f"qk{mb}", name=f"qk{mb}")
              for mb in range(6)]
        vaug = vpool.tile([128, 4, 8, 33], BF16, tag="vaug")

        def qk_chunk(mb):
            def go():
                ps = mmout.tile([128, 512], F32, tag="mm", name="qkps")
                nc.tensor.matmul(
                    ps, wqk_sb[:, :, 128 * mb:128 * mb + 128], xw8,
                    start=True, stop=True, perf_mode=DR)
                nc.vector.tensor_scalar(
                    qk[mb][:], ps, 1.0 / MSC, bq_sb[:, mb:mb + 1],
                    Mult, Add)
            return go

        def v_chunk(tp):
            def go():
                if tp == 0:
                    nc.gpsimd.memset(vaug[:, :, :, 32:33], 1.0)
                ps = mmout.tile([128, 512], F32, tag="mm", name="vps")
                for half in range(2):
                    for cb in range(2):
                        nc.tensor.matmul(
                            ps[:, 256 * half:256 * half + 256],
                            xw16[:, cb,
                                 128 * (2 * tp + half):128 * (2 * tp + half) + 128],
                            wv_sb[:, cb, :],
                            start=(cb == 0), stop=(cb == 1))
                bvb = bass.AP(tensor=bv_sb.tensor, offset=bv_sb.offset,
                              ap=[[bv_sb.ap[0][0], 128], [0, 2],
                                  [32, 8], [1, 32]])
                nc.vector.tensor_add(
                    vaug[:, 2 * tp:2 * tp + 2, :, 0:32],
                    ps.rearrange("p (t h d) -> p t h d", t=2, h=8),
                    bvb)
            return go

        chunks = ([qk_chunk(mb) for mb in range(6)]
                  + [v_chunk(0), v_chunk(1)])
        return qk, vaug, chunks

    def new_atto():
        atto_lo = apool.tile([128, 4, 128], BF16, tag="attolo", name="attolo")
        atto_hi = apool.tile([128, 4, 128], BF16, tag="attohi", name="attohi")
        return [atto_lo, atto_hi]

    def stage_head(h, qk, vaug, atto):
        """Emit scores + st-consumers (exp / quartic i1). Returns a closure
        with the rest of the head (quartic tail, AV, recip, normalize) so the
        caller can defer its emission by one head."""
        a, g = 32 * (h % 3), h // 3

        def st_tile(s):
            if qstpool is not None and (2 * h + s) in quart_set:
                return qstpool.tile([128, 2, 512], F32, tag="qst",
                                    name=f"qst{h}_{s}")
            return stpool.tile([128, 2, 512], F32, tag="st",
                               name=f"st{h}_{s}")

        st = [st_tile(0), st_tile(1)]
        nsc = 1 if os.environ.get("KERNEL_SC_CUT", "0") == "1" else 4
        for jb in range(nsc):
            nc.tensor.matmul(
                st[jb // 2][:, jb % 2, :],
                qk[3 + g][a:a + 32, 128 * jb:128 * jb + 128],
                qk[g][a:a + 32, :],
                start=True, stop=True)
        if nsc == 1:
            st[1] = st[0]
        PHDT = FP8 if os.environ.get("KERNEL_PH_FP8", "0") == "1" else BF16
        ph = ppool.tile([128, 4, 512], PHDT, tag="ph")
        us = [None, None]
        if os.environ.get("KERNEL_NO_EXP", "0") == "1":
            nc.gpsimd.memset(ph[:, 0:2, :], 0.002)
            nc.gpsimd.memset(ph[:, 2:4, :], 0.002)
        else:
            for s in range(2):
                if (2 * h + s) in quart_set:
                    # quartic exp substitute, stage i1: u = a*(S/64) + b
                    # (frees the PSUM st tile; the rest is SBUF-only)
                    u = upool.tile([128, 2, 512], BF16, tag="u")
                    nc.vector.tensor_scalar(u, st[s], QA / MSC, QB, Mult, Add)
                    us[s] = u
                else:
                    nc.scalar.activation(out=ph[:, 2 * s:2 * s + 2, :],
                                         in_=st[s], func=Exp, scale=1.0 / MSC)

        def rest():
            for s in range(2):
                if us[s] is not None:
                    # ((a*S+b)^2 + c)^2 * sl^2
                    u = us[s]
                    wt = wpool.tile([128, 2, 512], BF16, tag="w")
                    nc.vector.tensor_mul(wt, u, u)
                    qt = qpool.tile([128, 2, 512], BF16, tag="q")
                    nc.vector.tensor_scalar(qt, wt, QC, QSL, Add, Mult)
                    nc.vector.tensor_mul(ph[:, 2 * s:2 * s + 2, :], qt, qt)
            av = avpool.tile([128, 4, 33], F32, tag="avtx")
            njb = 1 if os.environ.get("KERNEL_AV_CUT", "0") == "1" else 4
            for qb in range(4):
                for jb in range(njb):
                    nc.tensor.matmul(
                        av[:, qb, :],
                        ph[:, jb, 128 * qb:128 * qb + 128],
                        vaug[:, jb, h, :],
                        start=(jb == 0), stop=(jb == njb - 1))
            rh = recpool.tile([128, 4], F32, tag="rec")
            if os.environ.get("KERNEL_RECIP", "exact") == "approx":
                nc.vector.reciprocal_approx_fast(out=rh, in_=av[:, :, 32])
            else:
                nc.vector.reciprocal(rh, av[:, :, 32])
            rb = bass.AP(tensor=rh.tensor, offset=rh.offset,
                         ap=[[rh.ap[0][0], 128], [rh.ap[1][0], 4], [0, 32]])
            nc.vector.tensor_mul(
                atto[h // 4][:, :, 32 * (h % 4):32 * (h % 4) + 32],
                av[:, :, 0:32], rb)

        return rest

    def stage_tail(iv, atto):
        at = atpool.tile([128, 2, 512], BF16, tag="at")
        for cb in range(2):
            tx = avpool.tile([128, 4, 128], BF16, tag="avtx")
            for tb in range(4):
                nc.tensor.transpose(
                    tx[:, tb, :], atto[cb][:, tb, :], ident)
            nc.vector.tensor_copy(at[:, cb, :], tx)
        for mb in range(2):
            ps = mmout.tile([128, 512], F32, tag="mm")
            for cb in range(2):
                nc.tensor.matmul(
                    ps, wp_sb[:, cb, 128 * mb:128 * mb + 128], at[:, cb, :],
                    start=(cb == 0), stop=(cb == 1))
            fin = finpool.tile([128, 512], F32, tag="fin")
            nc.vector.tensor_scalar_add(fin, ps, bp_sb[:, mb:mb + 1])
            nc.sync.dma_start(out=ot_d[iv, :, mb, :], in_=fin)

    return stage_qkv, new_atto, stage_head, stage_tail


def build_nc(n_windows=W, repeat=None):
    import concourse.bass as bass
    import concourse.tile as tile
    from concourse import bacc, mybir
    from concourse.masks import make_identity

    if repeat is None:
        repeat = int(os.environ.get("KERNEL_REPEAT", "1"))

    F32 = mybir.dt.float32
    BF16 = mybir.dt.bfloat16
    FP8 = mybir.dt.float8e4

    nc = bacc.Bacc("TRN2", target_bir_lowering=False, debug=False,
                   num_devices=N_CORES)
    xt8_d = nc.dram_tensor("xt8", [n_windows, 128, 2, 512], FP8,
                           kind="ExternalInput").ap()
    xt16_d = nc.dram_tensor("xt16", [n_windows, 128, 2, 512], BF16,
                            kind="ExternalInput").ap()
    wqk_d = nc.dram_tensor("wqk", [128, 2, 768], FP8,
                           kind="ExternalInput").ap()
    wv_d = nc.dram_tensor("wv", [128, 2, 256], BF16,
                          kind="ExternalInput").ap()
    bq_d = nc.dram_tensor("bq", [128, 6], F32, kind="ExternalInput").ap()
    bv_d = nc.dram_tensor("bv", [128, 256], F32, kind="ExternalInput").ap()
    wp_d = nc.dram_tensor("wp", [128, 2, 256], BF16, kind="ExternalInput").ap()
    bp_d = nc.dram_tensor("bp", [128, 2], F32, kind="ExternalInput").ap()
    ot_d = nc.dram_tensor("ot", [n_windows, 128, 2, 512], F32,
                          kind="ExternalOutput").ap()

    with tile.TileContext(nc) as tc, ExitStack() as ctx:
        persist = ctx.enter_context(tc.tile_pool(name="persist", bufs=1))
        xpool = ctx.enter_context(tc.tile_pool(name="xpool", bufs=3))
        x16pool = ctx.enter_context(tc.tile_pool(name="x16pool", bufs=3))
        qkpool = ctx.enter_context(tc.tile_pool(name="qkpool", bufs=3))
        vpool = ctx.enter_context(tc.tile_pool(name="vpool", bufs=3))
        ppool = ctx.enter_context(tc.tile_pool(name="ppool", bufs=3))
        # PSUM budget is 8 banks of 2KB/partition:
        #   st 2x2 + mmout 2x1 + av/tx 2x1 = 8 (KERNEL_QST=0)
        #   st 2x2 + mmout 1 + av/tx 1 + qst 1x2 = 8 (KERNEL_QST=1)
        use_qst = (os.environ.get("KERNEL_QST", "1") == "1"
                   and int(os.environ.get("KERNEL_NQUART", "1")) > 0)
        st_bufs = int(os.environ.get("KERNEL_ST_BUFS", "2"))
        mm_bufs = int(os.environ.get("KERNEL_MM_BUFS", "1" if use_qst else "2"))
        av_bufs = int(os.environ.get("KERNEL_AV_BUFS", "1" if use_qst else "2"))
        stpool = ctx.enter_context(tc.tile_pool(
            name="stpool", bufs=st_bufs, space="PSUM"))
        mmout = ctx.enter_context(tc.tile_pool(
            name="mmout", bufs=mm_bufs, space="PSUM"))
        avpool = ctx.enter_context(tc.tile_pool(
            name="avpool", bufs=av_bufs, space="PSUM"))
        qstpool = ctx.enter_context(tc.tile_pool(
            name="qstpool", bufs=1, space="PSUM")) if use_qst else None
        recpool = ctx.enter_context(tc.tile_pool(name="recpool", bufs=8))
        apool = ctx.enter_context(tc.tile_pool(name="apool", bufs=3))
        atpool = ctx.enter_context(tc.tile_pool(name="atpool", bufs=3))
        finpool = ctx.enter_context(tc.tile_pool(name="finpool", bufs=4))
        upool = ctx.enter_context(tc.tile_pool(name="upool", bufs=2))
        wpool = ctx.enter_context(tc.tile_pool(name="wpool", bufs=2))
        qpool = ctx.enter_context(tc.tile_pool(name="qpool", bufs=2))

        wqk_sb = persist.tile([128, 2, 768], FP8, tag="wqk")
        nc.sync.dma_start(out=wqk_sb, in_=wqk_d)
        wv_sb = persist.tile([128, 2, 256], BF16, tag="wv")
        nc.sync.dma_start(out=wv_sb, in_=wv_d)
        bq_sb = persist.tile([128, 6], F32, tag="bq")
        nc.sync.dma_start(out=bq_sb, in_=bq_d)
        bv_sb = persist.tile([128, 256], F32, tag="bv")
        nc.sync.dma_start(out=bv_sb, in_=bv_d)
        wp_sb = persist.tile([128, 2, 256], BF16, tag="wp")
        nc.sync.dma_start(out=wp_sb, in_=wp_d)
        bp_sb = persist.tile([128, 2], F32, tag="bp")
        nc.sync.dma_start(out=bp_sb, in_=bp_d)
        ident = persist.tile([128, 128], BF16, tag="id")
        make_identity(nc, ident)

        pools = (xpool, x16pool, qkpool, vpool, ppool, stpool, mmout,
                 avpool, recpool, apool, atpool, finpool, upool, wpool, qpool,
                 qstpool)
        consts = (wqk_sb, wv_sb, bq_sb, bv_sb, wp_sb, bp_sb, ident)
        stage_qkv, new_atto, stage_head, stage_tail = make_stages(
            nc, pools, consts, xt8_d, xt16_d, ot_d)

        def full_pass():
            defer = os.environ.get("KERNEL_DEFER", "1") == "1"
            interleave = os.environ.get("KERNEL_QKV_INTERLEAVE", "1") == "1"
            rest_q = []

            def push(r):
                if not defer:
                    r()
                    return
                rest_q.append(r)
                if len(rest_q) > 1:
                    rest_q.pop(0)()

            qk, vaug, chunks = stage_qkv(0)
            for c in chunks:
                c()
            for w in range(n_windows):
                atto = new_atto()
                nxt = stage_qkv(w + 1) if w + 1 < n_windows else None
                nchunks = nxt[2] if nxt else []
                for h in range(H):
                    push(stage_head(h, qk, vaug, atto))
                    if interleave and h < len(nchunks):
                        nchunks[h]()
                for c in (nchunks[H:] if interleave else nchunks):
                    c()
                while rest_q:
                    rest_q.pop(0)()
                stage_tail(w, atto)
                if nxt:
                    qk, vaug = nxt[0], nxt[1]

        body_passes = int(os.environ.get("KERNEL_BODY_PASSES", "1"))
        if repeat > 1:
            def rep_body(r):
                for _ in range(body_passes):
                    full_pass()
            tc.For_i_unrolled(0, repeat, 1, rep_body, max_unroll=1)
        else:
            full_pass()

    nc.compile()
    return nc


def prep_inputs(x, qkv_w, qkv_b, proj_w, proj_b, n_windows_per_core=W,
                n_cores=N_CORES):
    """Shard + lay out inputs for the per-core DRAM parameters."""
    from concourse import mybir

    FP8NP = mybir.dt.np(mybir.dt.float8e4)
    x = np.asarray(x, dtype=np.float32)
    qkv_w = np.asarray(qkv_w, dtype=np.float32)
    qkv_b = np.asarray(qkv_b, dtype=np.float32)
    proj_w = np.asarray(proj_w, dtype=np.float32)
    proj_b = np.asarray(proj_b, dtype=np.float32)

    sc = HD ** -0.5
    # q rows scaled by sc*QSC, k rows by KSC (fp8 exponent centering); the
    # fp8 matmuls run a further MSC hotter (folded into the weights), the
    # evacuation rescales by 1/MSC and adds the (sc*QSC / KSC)-scaled biases.
    qkv_w_s = qkv_w.copy()
    qkv_w_s[:C] *= sc * QSC * MSC
    qkv_w_s[C:2 * C] *= KSC * MSC
    qkv_b_s = qkv_b.copy()
    qkv_b_s[:C] *= sc * QSC
    qkv_b_s[C:2 * C] *= KSC

    # q/k feature blocks: 3 heads (96 feats) per 128-col block, zero padded,
    # so every head starts at partition offset 0/32/64.
    qpad = np.zeros((384, C), np.float32)
    kpad = np.zeros((384, C), np.float32)
    bqpad = np.zeros(768, np.float32)
    for b in range(3):
        lo, hi = 96 * b, min(96 * b + 96, C)
        qpad[128 * b:128 * b + hi - lo] = qkv_w_s[lo:hi]
        kpad[128 * b:128 * b + hi - lo] = qkv_w_s[C + lo:C + hi]
        bqpad[128 * b:128 * b + hi - lo] = qkv_b_s[lo:hi]
        bqpad[384 + 128 * b:384 + 128 * b + hi - lo] = qkv_b_s[C + lo:C + hi]
    wqk_all = np.concatenate([qpad, kpad], axis=0)  # [768, C]
    wqk = np.ascontiguousarray(
        wqk_all.reshape(768, 2, 128).transpose(2, 1, 0)).astype(FP8NP)
    wv = np.ascontiguousarray(
        qkv_w_s[2 * C:].reshape(C, 2, 128).transpose(2, 1, 0)
    ).astype(ml_dtypes.bfloat16)
    bq = np.ascontiguousarray(bqpad.reshape(6, 128).T)
    bv = np.ascontiguousarray(np.broadcast_to(qkv_b[2 * C:], (128, C)))
    wp = np.ascontiguousarray(
        proj_w.reshape(C, 2, 128).transpose(2, 1, 0)).astype(ml_dtypes.bfloat16)
    bp = np.ascontiguousarray(proj_b.reshape(2, 128).T)

    in_maps = []
    for c in range(n_cores):
        xs = x[c * n_windows_per_core:(c + 1) * n_windows_per_core]
        xt = np.ascontiguousarray(
            xs.reshape(n_windows_per_core, N, 2, 128).transpose(0, 3, 2, 1))
        in_maps.append(
            {"xt8": xt.astype(FP8NP), "xt16": xt.astype(ml_dtypes.bfloat16),
             "wqk": wqk, "wv": wv, "bq": bq, "bv": bv, "wp": wp, "bp": bp})
    return in_maps


def assemble_output(results, n_windows_per_core=W, n_cores=N_CORES):
    outs = []
    for c in range(n_cores):
        ot = results[c]["ot"]  # [W, 128, 2, 512]
        y = ot.transpose(0, 3, 2, 1).reshape(n_windows_per_core, N, C)
        outs.append(y)
    return np.ascontiguousarray(np.concatenate(outs, axis=0), dtype=np.float32)


_NC_CACHE = {}
LAST_EXEC_TIME_NS = None


def kernel(x, qkv_w, qkv_b, proj_w, proj_b):
    global LAST_EXEC_TIME_NS
    from concourse.bass_utils import run_bass_kernel_spmd

    if "nc" not in _NC_CACHE:
        _NC_CACHE["nc"] = build_nc(W, repeat=1)
    nc = _NC_CACHE["nc"]

    in_maps = prep_inputs(x, qkv_w, qkv_b, proj_w, proj_b)
    res = run_bass_kernel_spmd(nc, in_maps, core_ids=list(range(N_CORES)))
    LAST_EXEC_TIME_NS = res.exec_time_ns
    return assemble_output(res.results)
